# revision 10
# baseline (speedup 1.0000x reference)
"""Trainium2 Bass kernel for the STFT patch-dispatch loss (bf16 pipeline).

Per signal row x[262144] (fp32):
  reflect-pad -> blocks V[r=256, m=1028] via PE transpose (f32r, evac->bf16)
  folds on DVE (bf16 2x):  vp/vm = V_m +- V_{m+2};  vpp/vq = vp_t +- vp_{t+1}
  DFT: 24 bf16 matmul passes (4 freq classes x re/im, radix-4 recombination
  pre-folded), nyquist as a 1-row pass.  |X| = sqrt(re^2+im^2): squares ride
  the PSUM evacuation (ACT re / Pool im), add on DVE, sqrt on ACT.
Patch stage per batch row (bf16):
  d = a-b (DVE 2x), |d| via uint16 bitand 0x7fff (DVE 4x), t-window sums as
  a pairwise add tree (Pool r1/r2, DVE r3/r4), freq contraction via ones4
  matmul into PSUM [32, 3*65], DMA'd straight to DRAM.  Nyquist row goes
  through small PE transposes + a ones16 matmul -> [8, 27].
Host: assemble [33,65] patch sums, top-k mask + final scalar reductions.
"""
import numpy as np

import concourse.bass as bass
import concourse.bacc as bacc
import concourse.mybir as mybir
from concourse import tile

dt = mybir.dt
Alu = mybir.AluOpType
Act = mybir.ActivationFunctionType

B, L = 16, 262144
NCORES = 8
RPC = B // NCORES
NFFT, HOP, PS = 1024, 256, 16
PAD = NFFT // 2
LP = L + 2 * PAD
NBLK = LP // HOP            # 1028
T = 1 + (LP - NFFT) // HOP  # 1025
TP = 1040                   # t padded to 65 windows of 16
NF = 513
NPF, NPT = 33, 65
KSEL = max(1, int(NPF * NPT * 0.3))
D_RANGES = [(0, 512), (512, 1024), (1024, 1025)]


def _consts():
    from ml_dtypes import bfloat16
    r = np.arange(256)
    wc = np.empty((256, 512), np.float32)
    ws = np.empty((256, 512), np.float32)
    for c in range(4):
        k = 4 * np.arange(128) + c
        ang = 2.0 * np.pi * np.outer(r, k) / NFFT
        wc[:, 128 * c:128 * (c + 1)] = np.cos(ang)
        ws[:, 128 * c:128 * (c + 1)] = -np.sin(ang)
    # negated c1/c3 blocks for the 4-term odd-class matmuls
    wcn = np.concatenate([-wc[:, 128:256], -wc[:, 384:512]], axis=1)
    wsn = np.concatenate([-ws[:, 128:256], -ws[:, 384:512]], axis=1)
    wn = np.where(r % 2 == 0, 1.0, -1.0).astype(np.float32).reshape(256, 1)
    ones4 = (np.arange(128)[:, None] // 4 == np.arange(32)[None, :])
    ones16 = (np.arange(128)[:, None] // 16 == np.arange(8)[None, :])
    bf = lambda a: np.asarray(a, dtype=bfloat16)
    out = {
        "wc0": bf(wc[:128]), "wc1": bf(wc[128:]),
        "ws0": bf(ws[:128]), "ws1": bf(ws[128:]),
        "wcn0": bf(wcn[:128]), "wcn1": bf(wcn[128:]),
        "wsn0": bf(wsn[:128]), "wsn1": bf(wsn[128:]),
        "wn0": bf(wn[:128]), "wn1": bf(wn[128:]),
        "ones4": bf(ones4.astype(np.float32)),
        "ones16": bf(ones16.astype(np.float32)),
        "identr": np.eye(128, dtype=np.float32),
        "identb": bf(np.eye(128, dtype=np.float32)),
    }
    return out


CONST_SPECS = {
    "identr": ([128, 128], dt.float32r), "identb": ([128, 128], dt.bfloat16),
    "wc0": ([128, 512], dt.bfloat16), "wc1": ([128, 512], dt.bfloat16),
    "ws0": ([128, 512], dt.bfloat16), "ws1": ([128, 512], dt.bfloat16),
    "wcn0": ([128, 256], dt.bfloat16), "wcn1": ([128, 256], dt.bfloat16),
    "wsn0": ([128, 256], dt.bfloat16), "wsn1": ([128, 256], dt.bfloat16),
    "wn0": ([128, 1], dt.bfloat16), "wn1": ([128, 1], dt.bfloat16),
    "ones4": ([128, 32], dt.bfloat16), "ones16": ([128, 8], dt.bfloat16),
}


def build_nc(repeat=1):
    nc = bacc.Bacc("TRN2", target_bir_lowering=False, debug=False,
                   num_devices=NCORES)

    x_d = {s: nc.dram_tensor(f"x{s}", [RPC, L], dt.float32r,
                             kind="ExternalInput") for s in "stg"}
    c_d = {n: nc.dram_tensor(n, shp, cdt, kind="ExternalInput")
           for n, (shp, cdt) in CONST_SPECS.items()}
    osum_d = nc.dram_tensor("osum", [RPC, 32, 3 * NPT], dt.float32,
                            kind="ExternalOutput")
    onyq_d = nc.dram_tensor("onyq", [RPC, 8, 27], dt.float32,
                            kind="ExternalOutput")

    with tile.TileContext(nc) as tc:
        with (
            tc.tile_pool(name="const", bufs=1) as cp,
            tc.tile_pool(name="upool", bufs=2) as up,
            tc.tile_pool(name="vpool", bufs=2) as vp_,
            tc.tile_pool(name="fpool", bufs=2) as fp,
            tc.tile_pool(name="magp", bufs=1) as mp,
            tc.tile_pool(name="sqp", bufs=3) as sqp,
            tc.tile_pool(name="dpool", bufs=1) as dp,
            tc.tile_pool(name="tr_ps", bufs=1, space="PSUM") as tr_ps,
            tc.tile_pool(name="dft_ps", bufs=3, space="PSUM") as dft_ps,
            tc.tile_pool(name="sm_ps", bufs=1, space="PSUM") as sm_ps,
        ):
            C = {}
            for n, (shp, cdt) in CONST_SPECS.items():
                C[n] = cp.tile(shp, cdt, tag=n, name=f"c_{n}")
                nc.gpsimd.dma_start(C[n][:], c_d[n][:])

            # persistent pad-zeroed tiles
            nyb = cp.tile([2, 3, 1152], dt.float32r, tag="nyb", name="nyb")
            nc.gpsimd.memset(
                nyb[:].rearrange("p a b -> p (a b)").bitcast(dt.float32), 0.0)

            def load_u(s, b):
                """Issue the input DMAs for one signal row."""
                dmaq = nc.sync if s != "g" else nc.gpsimd
                u = up.tile([128, 8, 256], dt.float32r, tag="u", name="u",
                            bufs=3)
                dmaq.dma_start(
                    u[:], x_d[s][b:b + 1, :].rearrange(
                        "o (i p r) -> (o p) i r", i=8, r=256))
                scs = []
                for hi, lo in ((257, 1), (261887, 261631)):
                    sc = up.tile([2, 256], dt.float32r, tag="sc", name="sc",
                                 bufs=8)
                    dmaq.dma_start(sc[0:1, :], x_d[s][b:b + 1, hi:hi + 256])
                    dmaq.dma_start(sc[1:2, :], x_d[s][b:b + 1, lo:lo + 256])
                    scs.append(sc)
                return u, scs

            def v_build(loaded):
                """V [128, 2, 1028] bf16: V[r%128, r//128, m] = xp[256m+r]."""
                u, scs = loaded
                revs = []
                for sc in scs:
                    ur = up.tile([2, 256], dt.float32r, tag="ur", name="ur",
                                 bufs=4)
                    nc.vector.tensor_copy(ur[:], sc[0:2, 255::-1])
                    revs.append(ur)
                uh, ub = revs
                V = vp_.tile([128, 2, NBLK], dt.bfloat16, tag="V", name="V")
                groups = [
                    [(uh, 2), (u[:, 0, :], 128), (u[:, 1, :], 128),
                     (u[:, 2, :], 128)],                              # 386
                    [(u[:, 3, :], 128), (u[:, 4, :], 128),
                     (u[:, 5, :], 128), (u[:, 6, :], 128)],           # 512
                    [(u[:, 7, :], 128), (ub, 2)],                     # 130
                ]
                col = 0
                for pieces in groups:
                    width = sum(n for _, n in pieces)
                    for h in (0, 1):
                        tp = tr_ps.tile([128, 512], dt.float32r, tag="trp",
                                        name="tp")
                        off = 0
                        for uap, nr in pieces:
                            nc.tensor.transpose(
                                tp[:, off:off + nr],
                                uap[0:nr, 128 * h:128 * h + 128]
                                if nr != 128 else uap[:, 128 * h:128 * h + 128],
                                C["identr"][0:nr, 0:nr])
                            off += nr
                        nc.vector.tensor_copy(V[:, h, col:col + width],
                                              tp[:, 0:width])
                    col += width
                return V

            def folds(V, sig):
                vp = fp.tile([128, 2, NBLK - 2], dt.bfloat16, tag=f"vp{sig}", bufs=1)
                vm = fp.tile([128, 2, NBLK - 2], dt.bfloat16, tag=f"vm{sig}", bufs=1)
                nc.vector.tensor_add(vp[:], V[:, :, 0:NBLK - 2],
                                     V[:, :, 2:NBLK])
                nc.vector.tensor_sub(vm[:], V[:, :, 0:NBLK - 2],
                                     V[:, :, 2:NBLK])
                vpp = fp.tile([128, 2, T], dt.bfloat16, tag=f"vpp{sig}", bufs=1)
                vq = fp.tile([128, 2, T], dt.bfloat16, tag=f"vq{sig}", bufs=1)
                nc.vector.tensor_add(vpp[:], vp[:, :, 0:T], vp[:, :, 1:T + 1])
                nc.vector.tensor_sub(vq[:], vp[:, :, 0:T], vp[:, :, 1:T + 1])
                return vm, vpp, vq

            def chunk_mag(ops, c, sig, M):
                """DFT chunk c of signal sig -> mag in M[:, sig, 0:1025]."""
                vm, vpp, vq = ops
                cs = slice(128 * c, 128 * (c + 1))
                ns = slice(128 * (c // 2), 128 * (c // 2) + 128)
                if c % 2 == 0:
                    src = vpp if c == 0 else vq
                    terms_re = [("wc", cs, 0, src)]
                    terms_im = [("ws", cs, 0, src)]
                else:
                    if c == 1:
                        terms_re = [("wc", cs, 0, vm), ("ws", cs, 1, vm)]
                        terms_im = [("ws", cs, 0, vm), ("wcn", ns, 1, vm)]
                    else:
                        terms_re = [("wc", cs, 0, vm), ("wsn", ns, 1, vm)]
                        terms_im = [("ws", cs, 0, vm), ("wc", cs, 1, vm)]
                for lo, hi in ((0, 512), (512, 1024)):
                    ps = dft_ps.tile([128, 1024], dt.float32, tag="dftp",
                                     name="ps")
                    for half, terms in ((0, terms_re), (1, terms_im)):
                        nmm = 2 * len(terms)
                        k = 0
                        for wname, wsl, shift, srct in terms:
                            for h in (0, 1):
                                nc.tensor.matmul(
                                    ps[:, 512 * half:512 * half + hi - lo],
                                    C[wname + str(h)][:, wsl],
                                    srct[:, h, lo + shift:hi + shift],
                                    start=(k == 0), stop=(k == nmm - 1))
                                k += 1
                    sq2 = sqp.tile([128, 1024], dt.bfloat16, tag="sq2",
                                   name="sq2")
                    nc.scalar.activation(sq2[:], ps[:], Act.Square)
                    nc.gpsimd.tensor_add(M[:, sig, lo:hi], sq2[:, 0:512],
                                         sq2[:, 512:1024])
                # tail column t=1024
                pt = sm_ps.tile([128, 512], dt.float32, tag="nyp", name="pt")
                for half, terms in ((0, terms_re), (1, terms_im)):
                    nmm = 2 * len(terms)
                    k = 0
                    for wname, wsl, shift, srct in terms:
                        for h in (0, 1):
                            nc.tensor.matmul(
                                pt[:, half:half + 1],
                                C[wname + str(h)][:, wsl],
                                srct[:, h, 1024 + shift:1025 + shift],
                                start=(k == 0), stop=(k == nmm - 1))
                            k += 1
                sqt = sqp.tile([128, 2], dt.bfloat16, tag="sqt", name="sqt",
                               bufs=6)
                nc.scalar.activation(sqt[:], pt[:, 0:2], Act.Square)
                nc.gpsimd.tensor_add(M[:, sig, 1024:1025], sqt[:, 0:1],
                                     sqt[:, 1:2])
                nc.scalar.activation(M[:, sig, 0:T], M[:, sig, 0:T],
                                     Act.Sqrt)

            def nyq(ops, sig):
                """Raw X[512] -> nyb[0, sig, 0:1025] (f32r)."""
                vm, vpp, vq = ops
                for lo, hi in D_RANGES:
                    pn = sm_ps.tile([128, 512], dt.float32, tag="nyp",
                                    name="pn", bufs=1)
                    nc.tensor.matmul(pn[0:1, 0:hi - lo], C["wn0"][:, 0:1],
                                     vpp[:, 0, lo:hi], start=True, stop=False)
                    nc.tensor.matmul(pn[0:1, 0:hi - lo], C["wn1"][:, 0:1],
                                     vpp[:, 1, lo:hi], start=False, stop=True)
                    nc.scalar.copy(nyb[0:1, sig, lo:hi],
                                   pn[0:1, 0:hi - lo])

            def patch_chunk(c, M, t4s):
                """Per-chunk patch: subs, |.|, window add-tree."""
                for j, (sa, sb, sq) in enumerate(
                        ((0, 2, False), (1, 2, False), (0, 1, True))):
                    d = dp.tile([128, TP], dt.bfloat16, tag=f"d{j}",
                                name=f"d{j}", bufs=2)
                    nc.gpsimd.tensor_sub(d[:], M[:, sa, :], M[:, sb, :])
                    if sq:
                        nc.vector.tensor_mul(d[:], d[:], d[:])
                    else:
                        du = d[:].bitcast(dt.uint16)
                        nc.vector.tensor_scalar(du, du, 0x7FFF, None,
                                                Alu.bitwise_and)
                    d4 = d[:].rearrange("p (w e) -> p w e", e=16)
                    t1 = dp.tile([128, NPT, 8], dt.bfloat16, tag=f"t1{j}",
                                 bufs=2)
                    nc.gpsimd.tensor_add(t1[:], d4[:, :, 0:8],
                                         d4[:, :, 8:16])
                    t2 = dp.tile([128, NPT, 4], dt.bfloat16, tag=f"t2{j}",
                                 bufs=2)
                    nc.gpsimd.tensor_add(t2[:], t1[:, :, 0:4], t1[:, :, 4:8])
                    t3 = dp.tile([128, NPT, 2], dt.bfloat16, tag=f"t3{j}",
                                 bufs=2)
                    nc.vector.tensor_add(t3[:], t2[:, :, 0:2], t2[:, :, 2:4])
                    nc.vector.tensor_add(t4s[j][:, c],
                                         t3[:, :, 0:1], t3[:, :, 1:2])

            def row_finish(b, t4s):
                po = sm_ps.tile([128, 512], dt.float32, tag="nyp", name="po")
                for j in range(3):
                    for c in range(4):
                        nc.tensor.matmul(po[0:32, NPT * j:NPT * (j + 1)],
                                         C["ones4"][:], t4s[j][:, c, :, 0],
                                         start=(c == 0), stop=(c == 3))
                outm = dp.tile([32, 3 * NPT], dt.float32, tag="outm",
                               name="outm", bufs=2)
                nc.scalar.copy(outm[:], po[0:32, 0:3 * NPT])
                nc.gpsimd.dma_start(
                    osum_d[b:b + 1].rearrange("o q w -> (o q) w"), outm[:])
                # nyquist row: 27 column transposes -> [128, 27] (col=3g+sig)
                nyt_full = tr_ps.tile([128, 512], dt.float32r, tag="trp",
                                      name="nytp")
                nyt_ps = nyt_full[:, 0:54]
                for g in range(9):
                    for sig in range(3):
                        col = 2 * (3 * g + sig)
                        nc.tensor.transpose(
                            nyt_ps[:, col:col + 2],
                            nyb[0:2, sig, 128 * g:128 * g + 128],
                            C["identr"][0:2, 0:2])
                nyt = dp.tile([128, 54], dt.bfloat16, tag="nyt", name="nyt")
                nc.scalar.copy(nyt[:], nyt_ps)
                nytu = nyt[:].bitcast(dt.uint16)
                nc.vector.tensor_scalar(nytu, nytu, 0x7FFF, None,
                                        Alu.bitwise_and)
                dn = dp.tile([128, 27], dt.bfloat16, tag="dn", name="dn")
                for j, (sa, sb, sq) in enumerate(
                        ((0, 2, False), (1, 2, False), (0, 1, True))):
                    nyt3 = nyt[:].rearrange("p (g s e) -> p g s e", s=3, e=2)
                    dn3 = dn[:].rearrange("p (g s) -> p g s", s=3)
                    nc.vector.tensor_sub(dn3[:, :, j:j + 1],
                                         nyt3[:, :, sa, 0:1],
                                         nyt3[:, :, sb, 0:1])
                    if sq:
                        nc.vector.tensor_mul(dn3[:, :, j:j + 1],
                                             dn3[:, :, j:j + 1],
                                             dn3[:, :, j:j + 1])
                dnu = dn[:].bitcast(dt.uint16)
                nc.vector.tensor_scalar(dnu, dnu, 0x7FFF, None,
                                        Alu.bitwise_and)
                nyo = po[0:8, 200:227]
                nc.tensor.matmul(nyo, C["ones16"][:], dn[:],
                                 start=True, stop=True)
                outn = dp.tile([8, 27], dt.float32, tag="outn", name="outn",
                               bufs=2)
                nc.scalar.copy(outn[:], nyo)
                nc.gpsimd.dma_start(
                    onyq_d[b:b + 1].rearrange("o q w -> (o q) w"), outn[:])

            def row_process(b, loads, next_loads):
                # chunk-major: per chunk do all 3 signals then patch work
                ops = []
                for i, s in enumerate("stg"):
                    V = v_build(loads[s])
                    ops.append(folds(V, i))
                t4s = [dp.tile([128, 4, NPT, 1], dt.bfloat16, tag=f"t4{j}",
                               bufs=2, name=f"t4{j}") for j in range(3)]
                for c in range(4):
                    M = mp.tile([128, 3, TP], dt.bfloat16, tag="Mc",
                                name="Mc", bufs=2)
                    nc.vector.memset(M[:, :, 1025:TP], 0.0)
                    for sig in range(3):
                        chunk_mag(ops[sig], c, sig, M)
                    if c == 0:
                        for sig in range(3):
                            nyq(ops[sig], sig)
                        if next_loads is not None:
                            for s in "stg":
                                next_loads[s] = load_u(s, b + 1)
                    patch_chunk(c, M, t4s)
                row_finish(b, t4s)

            def body():
                loads = {s: load_u(s, 0) for s in "stg"}
                for b in range(RPC):
                    next_loads = {} if b + 1 < RPC else None
                    row_process(b, loads, next_loads)
                    loads = next_loads

            if repeat == 1:
                body()
            else:
                with tc.For_i(0, repeat, 1):
                    body()

    nc.compile()
    return nc


_NC_CACHE = {}


def _get_nc():
    if "nc" not in _NC_CACHE:
        _NC_CACHE["nc"] = build_nc()
    return _NC_CACHE["nc"]


def _run_on_cores(nc, in_maps):
    """Execute via cached PJRT callable (axon) with jit reuse."""
    from concourse.bass_utils import axon_active

    if not axon_active():
        from concourse.bass_utils import run_bass_kernel_spmd
        return run_bass_kernel_spmd(nc, in_maps,
                                    core_ids=list(range(NCORES))).results

    import jax
    from jax.sharding import Mesh, PartitionSpec
    from jax.experimental.shard_map import shard_map
    from concourse import bass2jax

    key = id(nc)
    if key not in _NC_CACHE.setdefault("jit", {}):
        bass2jax.install_neuronx_cc_hook()
        part_name = (nc.partition_id_tensor.name
                     if nc.partition_id_tensor else None)
        in_names, out_names, out_avals, zero_outs = [], [], [], []
        for alloc in nc.m.functions[0].allocations:
            if not isinstance(alloc, mybir.MemoryLocationSet):
                continue
            name = alloc.memorylocations[0].name
            if alloc.kind == "ExternalInput":
                if name != part_name:
                    in_names.append(name)
            elif alloc.kind == "ExternalOutput":
                shape = tuple(alloc.tensor_shape)
                dtype = mybir.dt.np(alloc.dtype)
                out_names.append(name)
                out_avals.append(jax.core.ShapedArray(shape, dtype))
                zero_outs.append(np.zeros(shape, dtype))
        n_params = len(in_names)
        all_names = in_names + out_names
        if part_name is not None:
            all_names = all_names + [part_name]

        def _body(*args):
            operands = list(args)
            if part_name is not None:
                operands.append(bass2jax.partition_id_tensor())
            outs = bass2jax._bass_exec_p.bind(
                *operands, out_avals=tuple(out_avals),
                in_names=tuple(all_names), out_names=tuple(out_names),
                lowering_input_output_aliases=(),
                sim_require_finite=True, sim_require_nnan=True, nc=nc)
            return tuple(outs)

        devices = jax.devices()[:NCORES]
        mesh = Mesh(np.asarray(devices), ("core",))
        n_outs = len(out_names)
        sharded = jax.jit(
            shard_map(_body, mesh=mesh,
                      in_specs=(PartitionSpec("core"),) * (n_params + n_outs),
                      out_specs=(PartitionSpec("core"),) * n_outs,
                      check_rep=False),
            donate_argnums=tuple(range(n_params, n_params + n_outs)),
            keep_unused=True)
        _NC_CACHE["jit"][key] = (sharded, in_names, out_names, out_avals,
                                 zero_outs)

    sharded, in_names, out_names, out_avals, zero_outs = _NC_CACHE["jit"][key]
    concat_in = [np.concatenate([m[n] for m in in_maps], axis=0)
                 for n in in_names]
    concat_zeros = [np.zeros((NCORES * z.shape[0], *z.shape[1:]), z.dtype)
                    for z in zero_outs]
    out_arrs = sharded(*concat_in, *concat_zeros)
    return [
        {n: np.asarray(out_arrs[i]).reshape(NCORES, *out_avals[i].shape)[c]
         for i, n in enumerate(out_names)}
        for c in range(NCORES)
    ]


def _assemble(osum, onyq):
    """osum [B,32,195], onyq [B,8,27] -> patch sums [B, 3, 33, 65]."""
    sums = np.zeros((B, 3, NPF, NPT), np.float32)
    main = osum.reshape(B, 32, 3, NPT).transpose(0, 2, 1, 3)
    sums[:, :, 0:32, :] = main
    nyv = onyq.reshape(B, 8, 9, 3)
    for w in range(NPT):
        g, i = w // 8, w % 8
        sums[:, :, 32, w] = nyv[:, i, g, :]
    return sums


def kernel(student_waveform, teacher_waveform, target_waveform,
           n_fft=1024, hop_length=256, patch_size=16):
    xs = np.ascontiguousarray(student_waveform, dtype=np.float32)
    xt = np.ascontiguousarray(teacher_waveform, dtype=np.float32)
    xg = np.ascontiguousarray(target_waveform, dtype=np.float32)

    nc = _get_nc()
    consts = _consts()
    in_maps = []
    for c in range(NCORES):
        m = {"xs": xs[RPC * c:RPC * (c + 1)],
             "xt": xt[RPC * c:RPC * (c + 1)],
             "xg": xg[RPC * c:RPC * (c + 1)]}
        m.update(consts)
        in_maps.append(m)

    results = _run_on_cores(nc, in_maps)

    osum = np.concatenate([r["osum"] for r in results], axis=0)
    onyq = np.concatenate([r["onyq"] for r in results], axis=0)
    sums = _assemble(osum, onyq).reshape(B, 3, NPF * NPT)
    inv = np.float32(1.0 / (PS * PS))
    err_s = sums[:, 0] * inv
    err_t = sums[:, 1] * inv
    pl = sums[:, 2] * inv
    kgs = err_s - err_t

    order = np.argsort(-kgs, axis=1, kind="stable")[:, :KSEL]
    mask = np.zeros_like(kgs)
    np.put_along_axis(mask, order, 1.0, axis=1)
    selected = (pl * mask).sum(axis=1, dtype=np.float32)
    count = np.maximum(mask.sum(axis=1, dtype=np.float32), 1.0)
    loss = np.float32(np.mean(selected / count, dtype=np.float32))
    sel_ratio = np.float32(mask.mean(dtype=np.float32))
    kgs_mean = np.float32(kgs.mean(dtype=np.float32))
    kgs_pos_ratio = np.float32((kgs > 0).mean(dtype=np.float32))
    return loss, sel_ratio, kgs_mean, kgs_pos_ratio


# revision 11
# speedup vs baseline: 1.1918x; 1.1918x over previous
"""Trainium2 Bass kernel for the STFT patch-dispatch loss (bf16 pipeline).

Per signal row x[262144] (fp32):
  reflect-pad -> blocks V[r=256, m=1028] via PE transpose (f32r, evac->bf16)
  folds on DVE (bf16 2x):  vp/vm = V_m +- V_{m+2};  vpp/vq = vp_t +- vp_{t+1}
  DFT: 24 bf16 matmul passes (4 freq classes x re/im, radix-4 recombination
  pre-folded), nyquist as a 1-row pass.  |X| = sqrt(re^2+im^2): squares ride
  the PSUM evacuation (ACT re / Pool im), add on DVE, sqrt on ACT.
Patch stage per batch row (bf16):
  d = a-b (DVE 2x), |d| via uint16 bitand 0x7fff (DVE 4x), t-window sums as
  a pairwise add tree (Pool r1/r2, DVE r3/r4), freq contraction via ones4
  matmul into PSUM [32, 3*65], DMA'd straight to DRAM.  Nyquist row goes
  through small PE transposes + a ones16 matmul -> [8, 27].
Host: assemble [33,65] patch sums, top-k mask + final scalar reductions.
"""
import numpy as np

import concourse.bass as bass
import concourse.bacc as bacc
import concourse.mybir as mybir
from concourse import tile

dt = mybir.dt
Alu = mybir.AluOpType
Act = mybir.ActivationFunctionType

B, L = 16, 262144
NCORES = 8
RPC = B // NCORES
NFFT, HOP, PS = 1024, 256, 16
PAD = NFFT // 2
LP = L + 2 * PAD
NBLK = LP // HOP            # 1028
T = 1 + (LP - NFFT) // HOP  # 1025
TP = 1040                   # t padded to 65 windows of 16
NF = 513
NPF, NPT = 33, 65
KSEL = max(1, int(NPF * NPT * 0.3))
D_RANGES = [(0, 512), (512, 1024), (1024, 1025)]


def _consts():
    from ml_dtypes import bfloat16
    r = np.arange(256)
    wc = np.empty((256, 512), np.float32)
    ws = np.empty((256, 512), np.float32)
    for c in range(4):
        k = 4 * np.arange(128) + c
        ang = 2.0 * np.pi * np.outer(r, k) / NFFT
        wc[:, 128 * c:128 * (c + 1)] = np.cos(ang)
        ws[:, 128 * c:128 * (c + 1)] = -np.sin(ang)
    # negated c1/c3 blocks for the 4-term odd-class matmuls
    wcn = np.concatenate([-wc[:, 128:256], -wc[:, 384:512]], axis=1)
    wsn = np.concatenate([-ws[:, 128:256], -ws[:, 384:512]], axis=1)
    wn = np.where(r % 2 == 0, 1.0, -1.0).astype(np.float32).reshape(256, 1)
    ones4 = (np.arange(128)[:, None] // 4 == np.arange(32)[None, :])
    ones16 = (np.arange(128)[:, None] // 16 == np.arange(8)[None, :])
    bf = lambda a: np.asarray(a, dtype=bfloat16)
    out = {
        "wc0": bf(wc[:128]), "wc1": bf(wc[128:]),
        "ws0": bf(ws[:128]), "ws1": bf(ws[128:]),
        "wcn0": bf(wcn[:128]), "wcn1": bf(wcn[128:]),
        "wsn0": bf(wsn[:128]), "wsn1": bf(wsn[128:]),
        "wn0": bf(wn[:128]), "wn1": bf(wn[128:]),
        "ones4": bf(ones4.astype(np.float32)),
        "ones16": bf(ones16.astype(np.float32)),
        "identr": np.eye(128, dtype=np.float32),
        "identb": bf(np.eye(128, dtype=np.float32)),
    }
    return out


CONST_SPECS = {
    "identr": ([128, 128], dt.float32r), "identb": ([128, 128], dt.bfloat16),
    "wc0": ([128, 512], dt.bfloat16), "wc1": ([128, 512], dt.bfloat16),
    "ws0": ([128, 512], dt.bfloat16), "ws1": ([128, 512], dt.bfloat16),
    "wcn0": ([128, 256], dt.bfloat16), "wcn1": ([128, 256], dt.bfloat16),
    "wsn0": ([128, 256], dt.bfloat16), "wsn1": ([128, 256], dt.bfloat16),
    "wn0": ([128, 1], dt.bfloat16), "wn1": ([128, 1], dt.bfloat16),
    "ones4": ([128, 32], dt.bfloat16), "ones16": ([128, 8], dt.bfloat16),
}


def build_nc(repeat=1):
    nc = bacc.Bacc("TRN2", target_bir_lowering=False, debug=False,
                   num_devices=NCORES)

    x_d = {s: nc.dram_tensor(f"x{s}", [RPC, L], dt.float32r,
                             kind="ExternalInput") for s in "stg"}
    c_d = {n: nc.dram_tensor(n, shp, cdt, kind="ExternalInput")
           for n, (shp, cdt) in CONST_SPECS.items()}
    osum_d = nc.dram_tensor("osum", [RPC, 32, 3 * NPT], dt.float32,
                            kind="ExternalOutput")
    onyq_d = nc.dram_tensor("onyq", [RPC, 8, 27], dt.float32,
                            kind="ExternalOutput")

    with tile.TileContext(nc) as tc:
        with (
            tc.tile_pool(name="const", bufs=1) as cp,
            tc.tile_pool(name="upool", bufs=2) as up,
            tc.tile_pool(name="vpool", bufs=2) as vp_,
            tc.tile_pool(name="fpool", bufs=2) as fp,
            tc.tile_pool(name="magp", bufs=1) as mp,
            tc.tile_pool(name="sqp", bufs=3) as sqp,
            tc.tile_pool(name="dpool", bufs=1) as dp,
            tc.tile_pool(name="tr_ps", bufs=1, space="PSUM") as tr_ps,
            tc.tile_pool(name="dft_ps", bufs=3, space="PSUM") as dft_ps,
            tc.tile_pool(name="sm_ps", bufs=1, space="PSUM") as sm_ps,
        ):
            C = {}
            for n, (shp, cdt) in CONST_SPECS.items():
                C[n] = cp.tile(shp, cdt, tag=n, name=f"c_{n}")
                nc.gpsimd.dma_start(C[n][:], c_d[n][:])

            # persistent pad-zeroed tiles
            nyb = cp.tile([2, 3, 1152], dt.float32r, tag="nyb", name="nyb")
            nc.gpsimd.memset(
                nyb[:].rearrange("p a b -> p (a b)").bitcast(dt.float32), 0.0)

            def load_u(s, b):
                """Issue the input DMAs for one signal row."""
                dmaq = nc.sync if s != "g" else nc.gpsimd
                u = up.tile([128, 8, 256], dt.float32r, tag="u", name="u",
                            bufs=3)
                dmaq.dma_start(
                    u[:], x_d[s][b:b + 1, :].rearrange(
                        "o (i p r) -> (o p) i r", i=8, r=256))
                scs = []
                for hi, lo in ((257, 1), (261887, 261631)):
                    sc = up.tile([2, 256], dt.float32r, tag="sc", name="sc",
                                 bufs=8)
                    dmaq.dma_start(sc[0:1, :], x_d[s][b:b + 1, hi:hi + 256])
                    dmaq.dma_start(sc[1:2, :], x_d[s][b:b + 1, lo:lo + 256])
                    scs.append(sc)
                return u, scs

            def v_build(loaded):
                """V [128, 2, 1028] bf16: V[r%128, r//128, m] = xp[256m+r]."""
                u, scs = loaded
                revs = []
                for sc in scs:
                    ur = up.tile([2, 256], dt.float32r, tag="ur", name="ur",
                                 bufs=4)
                    nc.vector.tensor_copy(ur[:], sc[0:2, 255::-1])
                    revs.append(ur)
                uh, ub = revs
                V = vp_.tile([128, 2, NBLK], dt.bfloat16, tag="V", name="V")
                groups = [
                    [(uh, 2), (u[:, 0, :], 128), (u[:, 1, :], 128),
                     (u[:, 2, :], 128)],                              # 386
                    [(u[:, 3, :], 128), (u[:, 4, :], 128),
                     (u[:, 5, :], 128), (u[:, 6, :], 128)],           # 512
                    [(u[:, 7, :], 128), (ub, 2)],                     # 130
                ]
                col = 0
                for pieces in groups:
                    width = sum(n for _, n in pieces)
                    for h in (0, 1):
                        tp = tr_ps.tile([128, 512], dt.float32r, tag="trp",
                                        name="tp")
                        off = 0
                        for uap, nr in pieces:
                            nc.tensor.transpose(
                                tp[:, off:off + nr],
                                uap[0:nr, 128 * h:128 * h + 128]
                                if nr != 128 else uap[:, 128 * h:128 * h + 128],
                                C["identr"][0:nr, 0:nr])
                            off += nr
                        nc.vector.tensor_copy(V[:, h, col:col + width],
                                              tp[:, 0:width])
                    col += width
                return V

            def folds(V, sig):
                vp = fp.tile([128, 2, NBLK - 2], dt.bfloat16, tag=f"vp{sig}", bufs=1)
                vm = fp.tile([128, 2, NBLK - 2], dt.bfloat16, tag=f"vm{sig}", bufs=1)
                nc.vector.tensor_add(vp[:], V[:, :, 0:NBLK - 2],
                                     V[:, :, 2:NBLK])
                nc.vector.tensor_sub(vm[:], V[:, :, 0:NBLK - 2],
                                     V[:, :, 2:NBLK])
                vpp = fp.tile([128, 2, T], dt.bfloat16, tag=f"vpp{sig}", bufs=1)
                vq = fp.tile([128, 2, T], dt.bfloat16, tag=f"vq{sig}", bufs=1)
                nc.vector.tensor_add(vpp[:], vp[:, :, 0:T], vp[:, :, 1:T + 1])
                nc.vector.tensor_sub(vq[:], vp[:, :, 0:T], vp[:, :, 1:T + 1])
                return vm, vpp, vq

            def chunk_mag(ops, c, sig, M):
                """DFT chunk c of signal sig -> mag in M[:, sig, 0:1025]."""
                vm, vpp, vq = ops
                cs = slice(128 * c, 128 * (c + 1))
                ns = slice(128 * (c // 2), 128 * (c // 2) + 128)
                if c % 2 == 0:
                    src = vpp if c == 0 else vq
                    terms_re = [("wc", cs, 0, src)]
                    terms_im = [("ws", cs, 0, src)]
                else:
                    if c == 1:
                        terms_re = [("wc", cs, 0, vm), ("ws", cs, 1, vm)]
                        terms_im = [("ws", cs, 0, vm), ("wcn", ns, 1, vm)]
                    else:
                        terms_re = [("wc", cs, 0, vm), ("wsn", ns, 1, vm)]
                        terms_im = [("ws", cs, 0, vm), ("wc", cs, 1, vm)]
                for lo, hi in ((0, 512), (512, 1024)):
                    ps = dft_ps.tile([128, 1024], dt.float32, tag="dftp",
                                     name="ps")
                    for half, terms in ((0, terms_re), (1, terms_im)):
                        nmm = 2 * len(terms)
                        k = 0
                        for wname, wsl, shift, srct in terms:
                            for h in (0, 1):
                                nc.tensor.matmul(
                                    ps[:, 512 * half:512 * half + hi - lo],
                                    C[wname + str(h)][:, wsl],
                                    srct[:, h, lo + shift:hi + shift],
                                    start=(k == 0), stop=(k == nmm - 1))
                                k += 1
                    sq2 = sqp.tile([128, 1024], dt.bfloat16, tag="sq2",
                                   name="sq2")
                    nc.scalar.activation(sq2[:], ps[:], Act.Square)
                    nc.vector.tensor_add(M[:, sig, lo:hi], sq2[:, 0:512],
                                         sq2[:, 512:1024])
                # tail column t=1024
                pt = sm_ps.tile([128, 512], dt.float32, tag="nyp", name="pt")
                for half, terms in ((0, terms_re), (1, terms_im)):
                    nmm = 2 * len(terms)
                    k = 0
                    for wname, wsl, shift, srct in terms:
                        for h in (0, 1):
                            nc.tensor.matmul(
                                pt[:, half:half + 1],
                                C[wname + str(h)][:, wsl],
                                srct[:, h, 1024 + shift:1025 + shift],
                                start=(k == 0), stop=(k == nmm - 1))
                            k += 1
                sqt = sqp.tile([128, 2], dt.bfloat16, tag="sqt", name="sqt",
                               bufs=6)
                nc.scalar.activation(sqt[:], pt[:, 0:2], Act.Square)
                nc.vector.tensor_add(M[:, sig, 1024:1025], sqt[:, 0:1],
                                     sqt[:, 1:2])
                nc.scalar.activation(M[:, sig, 0:T], M[:, sig, 0:T],
                                     Act.Sqrt)

            def nyq(ops, sig):
                """Raw X[512] -> nyb[0, sig, 0:1025] (f32r)."""
                vm, vpp, vq = ops
                for lo, hi in D_RANGES:
                    pn = sm_ps.tile([128, 512], dt.float32, tag="nyp",
                                    name="pn", bufs=1)
                    nc.tensor.matmul(pn[0:1, 0:hi - lo], C["wn0"][:, 0:1],
                                     vpp[:, 0, lo:hi], start=True, stop=False)
                    nc.tensor.matmul(pn[0:1, 0:hi - lo], C["wn1"][:, 0:1],
                                     vpp[:, 1, lo:hi], start=False, stop=True)
                    nc.scalar.copy(nyb[0:1, sig, lo:hi],
                                   pn[0:1, 0:hi - lo])

            def patch_chunk(c, M, t4s):
                """Per-chunk patch: subs, |.|, window add-tree."""
                for j, (sa, sb, sq) in enumerate(
                        ((0, 2, False), (1, 2, False), (0, 1, True))):
                    d = dp.tile([128, TP], dt.bfloat16, tag=f"d{j}",
                                name=f"d{j}", bufs=2)
                    nc.vector.tensor_sub(d[:], M[:, sa, :], M[:, sb, :])
                    if sq:
                        nc.vector.tensor_mul(d[:], d[:], d[:])
                    else:
                        du = d[:].bitcast(dt.uint16)
                        nc.vector.tensor_scalar(du, du, 0x7FFF, None,
                                                Alu.bitwise_and)
                    d4 = d[:].rearrange("p (w e) -> p w e", e=16)
                    t1 = dp.tile([128, NPT, 8], dt.bfloat16, tag=f"t1{j}",
                                 bufs=2)
                    nc.vector.tensor_add(t1[:], d4[:, :, 0:8],
                                         d4[:, :, 8:16])
                    t2 = dp.tile([128, NPT, 4], dt.bfloat16, tag=f"t2{j}",
                                 bufs=2)
                    nc.vector.tensor_add(t2[:], t1[:, :, 0:4], t1[:, :, 4:8])
                    t3 = dp.tile([128, NPT, 2], dt.bfloat16, tag=f"t3{j}",
                                 bufs=2)
                    nc.vector.tensor_add(t3[:], t2[:, :, 0:2], t2[:, :, 2:4])
                    nc.vector.tensor_add(t4s[j][:, c],
                                         t3[:, :, 0:1], t3[:, :, 1:2])

            def row_finish(b, t4s):
                po = sm_ps.tile([128, 512], dt.float32, tag="nyp", name="po")
                for j in range(3):
                    for c in range(4):
                        nc.tensor.matmul(po[0:32, NPT * j:NPT * (j + 1)],
                                         C["ones4"][:], t4s[j][:, c, :, 0],
                                         start=(c == 0), stop=(c == 3))
                outm = dp.tile([32, 3 * NPT], dt.float32, tag="outm",
                               name="outm", bufs=2)
                nc.scalar.copy(outm[:], po[0:32, 0:3 * NPT])
                nc.gpsimd.dma_start(
                    osum_d[b:b + 1].rearrange("o q w -> (o q) w"), outm[:])
                # nyquist row: 27 column transposes -> [128, 27] (col=3g+sig)
                nyt_full = tr_ps.tile([128, 512], dt.float32r, tag="trp",
                                      name="nytp")
                nyt_ps = nyt_full[:, 0:54]
                for g in range(9):
                    for sig in range(3):
                        col = 2 * (3 * g + sig)
                        nc.tensor.transpose(
                            nyt_ps[:, col:col + 2],
                            nyb[0:2, sig, 128 * g:128 * g + 128],
                            C["identr"][0:2, 0:2])
                nyt = dp.tile([128, 54], dt.bfloat16, tag="nyt", name="nyt")
                nc.scalar.copy(nyt[:], nyt_ps)
                nytu = nyt[:].bitcast(dt.uint16)
                nc.vector.tensor_scalar(nytu, nytu, 0x7FFF, None,
                                        Alu.bitwise_and)
                dn = dp.tile([128, 27], dt.bfloat16, tag="dn", name="dn")
                for j, (sa, sb, sq) in enumerate(
                        ((0, 2, False), (1, 2, False), (0, 1, True))):
                    nyt3 = nyt[:].rearrange("p (g s e) -> p g s e", s=3, e=2)
                    dn3 = dn[:].rearrange("p (g s) -> p g s", s=3)
                    nc.vector.tensor_sub(dn3[:, :, j:j + 1],
                                         nyt3[:, :, sa, 0:1],
                                         nyt3[:, :, sb, 0:1])
                    if sq:
                        nc.vector.tensor_mul(dn3[:, :, j:j + 1],
                                             dn3[:, :, j:j + 1],
                                             dn3[:, :, j:j + 1])
                dnu = dn[:].bitcast(dt.uint16)
                nc.vector.tensor_scalar(dnu, dnu, 0x7FFF, None,
                                        Alu.bitwise_and)
                nyo = po[0:8, 200:227]
                nc.tensor.matmul(nyo, C["ones16"][:], dn[:],
                                 start=True, stop=True)
                outn = dp.tile([8, 27], dt.float32, tag="outn", name="outn",
                               bufs=2)
                nc.scalar.copy(outn[:], nyo)
                nc.gpsimd.dma_start(
                    onyq_d[b:b + 1].rearrange("o q w -> (o q) w"), outn[:])

            def row_process(b, loads, next_loads):
                # chunk-major: per chunk do all 3 signals then patch work
                ops = []
                for i, s in enumerate("stg"):
                    V = v_build(loads[s])
                    ops.append(folds(V, i))
                t4s = [dp.tile([128, 4, NPT, 1], dt.bfloat16, tag=f"t4{j}",
                               bufs=2, name=f"t4{j}") for j in range(3)]
                for c in range(4):
                    M = mp.tile([128, 3, TP], dt.bfloat16, tag="Mc",
                                name="Mc", bufs=2)
                    nc.vector.memset(M[:, :, 1025:TP], 0.0)
                    for sig in range(3):
                        chunk_mag(ops[sig], c, sig, M)
                    if c == 0:
                        for sig in range(3):
                            nyq(ops[sig], sig)
                        if next_loads is not None:
                            for s in "stg":
                                next_loads[s] = load_u(s, b + 1)
                    patch_chunk(c, M, t4s)
                row_finish(b, t4s)

            def body():
                loads = {s: load_u(s, 0) for s in "stg"}
                for b in range(RPC):
                    next_loads = {} if b + 1 < RPC else None
                    row_process(b, loads, next_loads)
                    loads = next_loads

            if repeat == 1:
                body()
            else:
                with tc.For_i(0, repeat, 1):
                    body()

    nc.compile()
    return nc


_NC_CACHE = {}


def _get_nc():
    if "nc" not in _NC_CACHE:
        _NC_CACHE["nc"] = build_nc()
    return _NC_CACHE["nc"]


def _run_on_cores(nc, in_maps):
    """Execute via cached PJRT callable (axon) with jit reuse."""
    from concourse.bass_utils import axon_active

    if not axon_active():
        from concourse.bass_utils import run_bass_kernel_spmd
        return run_bass_kernel_spmd(nc, in_maps,
                                    core_ids=list(range(NCORES))).results

    import jax
    from jax.sharding import Mesh, PartitionSpec
    from jax.experimental.shard_map import shard_map
    from concourse import bass2jax

    key = id(nc)
    if key not in _NC_CACHE.setdefault("jit", {}):
        bass2jax.install_neuronx_cc_hook()
        part_name = (nc.partition_id_tensor.name
                     if nc.partition_id_tensor else None)
        in_names, out_names, out_avals, zero_outs = [], [], [], []
        for alloc in nc.m.functions[0].allocations:
            if not isinstance(alloc, mybir.MemoryLocationSet):
                continue
            name = alloc.memorylocations[0].name
            if alloc.kind == "ExternalInput":
                if name != part_name:
                    in_names.append(name)
            elif alloc.kind == "ExternalOutput":
                shape = tuple(alloc.tensor_shape)
                dtype = mybir.dt.np(alloc.dtype)
                out_names.append(name)
                out_avals.append(jax.core.ShapedArray(shape, dtype))
                zero_outs.append(np.zeros(shape, dtype))
        n_params = len(in_names)
        all_names = in_names + out_names
        if part_name is not None:
            all_names = all_names + [part_name]

        def _body(*args):
            operands = list(args)
            if part_name is not None:
                operands.append(bass2jax.partition_id_tensor())
            outs = bass2jax._bass_exec_p.bind(
                *operands, out_avals=tuple(out_avals),
                in_names=tuple(all_names), out_names=tuple(out_names),
                lowering_input_output_aliases=(),
                sim_require_finite=True, sim_require_nnan=True, nc=nc)
            return tuple(outs)

        devices = jax.devices()[:NCORES]
        mesh = Mesh(np.asarray(devices), ("core",))
        n_outs = len(out_names)
        sharded = jax.jit(
            shard_map(_body, mesh=mesh,
                      in_specs=(PartitionSpec("core"),) * (n_params + n_outs),
                      out_specs=(PartitionSpec("core"),) * n_outs,
                      check_rep=False),
            donate_argnums=tuple(range(n_params, n_params + n_outs)),
            keep_unused=True)
        _NC_CACHE["jit"][key] = (sharded, in_names, out_names, out_avals,
                                 zero_outs)

    sharded, in_names, out_names, out_avals, zero_outs = _NC_CACHE["jit"][key]
    concat_in = [np.concatenate([m[n] for m in in_maps], axis=0)
                 for n in in_names]
    concat_zeros = [np.zeros((NCORES * z.shape[0], *z.shape[1:]), z.dtype)
                    for z in zero_outs]
    out_arrs = sharded(*concat_in, *concat_zeros)
    return [
        {n: np.asarray(out_arrs[i]).reshape(NCORES, *out_avals[i].shape)[c]
         for i, n in enumerate(out_names)}
        for c in range(NCORES)
    ]


def _assemble(osum, onyq):
    """osum [B,32,195], onyq [B,8,27] -> patch sums [B, 3, 33, 65]."""
    sums = np.zeros((B, 3, NPF, NPT), np.float32)
    main = osum.reshape(B, 32, 3, NPT).transpose(0, 2, 1, 3)
    sums[:, :, 0:32, :] = main
    nyv = onyq.reshape(B, 8, 9, 3)
    for w in range(NPT):
        g, i = w // 8, w % 8
        sums[:, :, 32, w] = nyv[:, i, g, :]
    return sums


def kernel(student_waveform, teacher_waveform, target_waveform,
           n_fft=1024, hop_length=256, patch_size=16):
    xs = np.ascontiguousarray(student_waveform, dtype=np.float32)
    xt = np.ascontiguousarray(teacher_waveform, dtype=np.float32)
    xg = np.ascontiguousarray(target_waveform, dtype=np.float32)

    nc = _get_nc()
    consts = _consts()
    in_maps = []
    for c in range(NCORES):
        m = {"xs": xs[RPC * c:RPC * (c + 1)],
             "xt": xt[RPC * c:RPC * (c + 1)],
             "xg": xg[RPC * c:RPC * (c + 1)]}
        m.update(consts)
        in_maps.append(m)

    results = _run_on_cores(nc, in_maps)

    osum = np.concatenate([r["osum"] for r in results], axis=0)
    onyq = np.concatenate([r["onyq"] for r in results], axis=0)
    sums = _assemble(osum, onyq).reshape(B, 3, NPF * NPT)
    inv = np.float32(1.0 / (PS * PS))
    err_s = sums[:, 0] * inv
    err_t = sums[:, 1] * inv
    pl = sums[:, 2] * inv
    kgs = err_s - err_t

    order = np.argsort(-kgs, axis=1, kind="stable")[:, :KSEL]
    mask = np.zeros_like(kgs)
    np.put_along_axis(mask, order, 1.0, axis=1)
    selected = (pl * mask).sum(axis=1, dtype=np.float32)
    count = np.maximum(mask.sum(axis=1, dtype=np.float32), 1.0)
    loss = np.float32(np.mean(selected / count, dtype=np.float32))
    sel_ratio = np.float32(mask.mean(dtype=np.float32))
    kgs_mean = np.float32(kgs.mean(dtype=np.float32))
    kgs_pos_ratio = np.float32((kgs > 0).mean(dtype=np.float32))
    return loss, sel_ratio, kgs_mean, kgs_pos_ratio
